# revision 23
# baseline (speedup 1.0000x reference)
"""Trainium2 Bass kernel for a Code2Vec encoder.

Reference computation (per batch b of 512, contexts c of 200):
    s = node_emb[starts]; p = path_emb[paths]; e = node_emb[ends]
    ctx = concat(s, p, e)                      # [B, C, 384]
    h = tanh(ctx @ W.T)                        # [B, C, 384]
    scores = h @ a                             # [B, C, 1]
    attn = softmax(scores, axis=1)
    out = sum(h * attn, axis=1)                # [B, 384]

Sharding: data-parallel over batch across 8 NeuronCores (64 batches each).
Embedding tables are row-sharded per core by usage: each core's in_map holds
only the table rows that core actually references (<=25600 node rows,
<=12800 path rows per core), cast to bf16, with indices remapped to int16
local ids.  That bounds indices to the int16 range dma_gather requires.

Per-core layout: columns j = c*64 + b (c-major), 12800 columns in 100
chunks of 128.  Chunk k, partition-within-chunk p <-> (b = p % 64,
c = 2k + p // 64).
 - gather s/p/e via SWDGE dma_gather(transpose=True): one call per
   (table, group of 2560 columns) batches 2560 descriptors (~1.9us Pool
   prep each vs ~1us per 128 rows for indirect_dma_start) AND lands the
   data transposed in SBUF as [128 features, columns] = ready-made lhsT.
 - matmul per 128-column chunk (lhsT = gathered chunk per table, rhs =
   W.T rows, all bf16) accumulating the three 128-feature groups ->
   psum h [128 cols, 384v]
 - ACT tanh -> bf16 ht chunk; DVE tensor_tensor_reduce ht*a -> scores col
 - attention without the max-subtraction pass (scores are O(1), exp is
   safe): escore = exp(scores) on ACT, da[p, b] = diag[p, b]*escore[p]
   on ACT, then two PE matmuls accumulate o_ps[b, v] += da^T @ ht and
   oz_ps[b] += da^T @ 1 across all 100 chunks.  wsum matmuls trail the
   main loop by LAG chunks so PE never stalls on the ACT/DVE score chain.
 - epilogue: out = o_ps * reciprocal(oz_ps)  (the softmax normalization)
"""

import os
import numpy as np
import ml_dtypes

B, C, E = 512, 200, 128
NODES, PATHS, CV = 100000, 1000000, 384
NCORES = 8
BPC = B // NCORES          # 64 batches per core
R = C * BPC                # 12800 columns per core
P = 128
NCHUNK = R // P            # 100 chunks of 128 columns
# Gather group sizes in chunks: small first (fast pipeline fill), big in
# the middle (amortize the ~1us SWDGE launch per call), small last (short
# PE tail after the final gather).  All gathers stay on ONE SWDGE queue:
# transpose-mode gathers go through the shared XBAR, and concurrent
# transpose gathers on different queues corrupt each other (probed on HW).
GROUPS = (2, 28, 28, 28, 12, 2)
assert sum(GROUPS) == NCHUNK
NGROUP = len(GROUPS)
GOFF = [sum(GROUPS[:i]) for i in range(NGROUP)]  # first chunk of each group
GMAX = max(GROUPS)
IDXW_G = [g * P // 16 for g in GROUPS]           # idx words per group
IDXW = GMAX * P // 16                            # idx tile pitch (padded)
NQ = 1                     # SWDGE queues; queue q runs on Q7 cores (2q,2q+1)
UN = 2 * R                 # compact node table rows (worst case 25600)
UP = R                     # compact path table rows (worst case 12800)
EB = 10                    # exp batch: one ACT exp per EB chunks; wsum
                           # matmuls trail the score chain by one batch

_compiled = {}


def _build_program():
    import concourse.bass as bass
    import concourse.mybir as mybir
    import concourse.tile as tile
    from concourse.tile_rust import add_dep_helper
    from concourse.dve_ops import TENSOR_TENSOR_REDUCE
    from concourse.library_config import mlp

    f32 = mybir.dt.float32
    bf16 = mybir.dt.bfloat16
    i16 = mybir.dt.int16

    # 64 KiB SWDGE descriptor-ring carveout (default 16 KiB = 1024 descs
    # per queue; each 2560-column dma_gather pushes 2560 descriptors, and an
    # entry that can never fit wedges the Q7 desc-gen kernel forever).
    nc = bass.Bass(
        "TRN2", target_bir_lowering=False, debug=False,
        dynamic_dma_scratch_size=65536,
        num_swdge_queues=NQ,
    )

    # Suppress Tile's tail DMA-reset drain (19 sem waits — walrus allows 1
    # per instruction) and sem range-clear.  They only matter for relaunch
    # of the same loaded NEFF; every kernel() call loads a fresh NEFF.
    type(nc.gpsimd).dma_reset = lambda self, *a, **k: None
    type(nc.gpsimd).sem_clear = lambda self, *a, **k: None

    # Per-engine emission-order chaining (free same-engine ordering edges).
    # This pins the per-engine schedule to emission order so the
    # single-sync-wait budget of Matmult/Ldweights is respected by
    # construction: each instruction's older cross-engine deps are already
    # covered by the engine clock via earlier chained instructions.
    _prev = {}

    def chain(bi):
        eng = bi.ins.engine
        p = _prev.get(eng)
        if p is not None:
            add_dep_helper(bi.ins, p.ins, sync=False, reason="engine chain")
        _prev[eng] = bi
        return bi

    # DRAM I/O.  Compact per-core tables (bf16), padded to fixed shapes.
    nodec_d = nc.dram_tensor("nodec", [UN, E], bf16, kind="ExternalInput").ap()
    pathc_d = nc.dram_tensor("pathc", [UP, E], bf16, kind="ExternalInput").ap()
    wt_d = nc.dram_tensor("wt", [3 * E, CV], bf16, kind="ExternalInput").ap()
    abc_d = nc.dram_tensor("a_bc", [P, CV], bf16, kind="ExternalInput").ap()
    diag_d = nc.dram_tensor("diag", [P, BPC], bf16, kind="ExternalInput").ap()
    ones_d = nc.dram_tensor("ones", [P, 1], bf16, kind="ExternalInput").ap()
    idx_d = nc.dram_tensor(
        "idx", [P, 3, NGROUP, IDXW], i16, kind="ExternalInput"
    ).ap()
    out_d = nc.dram_tensor("out", [BPC, CV], f32, kind="ExternalOutput").ap()

    with tile.TileContext(nc) as tc:
        with (
            tc.tile_pool(name="const", bufs=1) as cpool,
            tc.tile_pool(name="gather", bufs=1) as gpool,
            tc.tile_pool(name="work", bufs=4) as wpool,
            tc.tile_pool(name="psum_h", bufs=3, space="PSUM") as ph_pool,
            tc.tile_pool(name="psum_o", bufs=1, space="PSUM") as po_pool,
        ):
            # dma_gather runs as a GPSIMD Q7 ucode kernel from the `mlp`
            # library; load it before the first gather.  (No other Pool
            # instruction in this program needs a different library.)
            chain(nc.gpsimd.load_library(mlp))

            # ---- constants ----
            wt_sb = cpool.tile([P, 3, CV], bf16, name="wt_sb")
            chain(nc.sync.dma_start(
                out=wt_sb[:], in_=wt_d.rearrange("(t p) v -> p t v", p=P)
            ))
            abc_sb = cpool.tile([P, CV], bf16, name="abc_sb")
            chain(nc.sync.dma_start(out=abc_sb[:], in_=abc_d[:]))
            diag_sb = cpool.tile([P, BPC], bf16, name="diag_sb")
            chain(nc.sync.dma_start(out=diag_sb[:], in_=diag_d[:]))
            ones_sb = cpool.tile([P, 1], bf16, name="ones_sb")
            chain(nc.sync.dma_start(out=ones_sb[:], in_=ones_d[:]))
            idx_sb = cpool.tile([P, 3, NGROUP, IDXW], i16, name="idx_sb")
            chain(nc.sync.dma_start(out=idx_sb[:], in_=idx_d[:]))

            # score columns (tanh outputs live in a short ring of chunk
            # tiles: the wsum matmul consumes chunk k only LAG chunks later,
            # and the 64 KiB DMA-scratch carveout wants the SBUF back)
            scores = cpool.tile([P, NCHUNK], f32, name="scores")
            escore = cpool.tile([P, NCHUNK], f32, name="escore")

            # One-time "touch" reads absorb the const-DMA waits on each
            # consuming engine (walrus allows only ONE sync wait on a
            # Matmult/Ldweights, so no PE instruction may need two).
            scr_ps = po_pool.tile([1, 8], f32, name="scr_ps")
            scr_sb = cpool.tile([1, 8], f32, name="scr_sb")
            scr_bf = cpool.tile([1, 8], bf16, name="scr_bf")

            def pe_touch(ap):
                return chain(nc.tensor.matmul(
                    out=scr_ps[0:1, 0:1], lhsT=ap, rhs=ap,
                    start=True, stop=True, skip_group_check=True,
                ))

            def rebase(tile_h, inst):
                # Rebase a tile's dep state onto `inst` so every later
                # reader deps on a same-engine instruction (no extra sem
                # wait) instead of the original DMA.
                tc.dep_state.clear_tensor_accesses(tile_h.tensor.name)
                tc.dep_state.set_after_insts(tile_h.tensor.name, inst.ins)

            tw = pe_touch(wt_sb[0:1, 0, 0:1])
            rebase(wt_sb, tw)
            to = pe_touch(ones_sb[0:1, 0:1])
            rebase(ones_sb, to)
            ta = chain(nc.vector.tensor_copy(scr_bf[0:1, 0:1], abc_sb[0:1, 0:1]))
            rebase(abc_sb, ta)
            td = chain(nc.scalar.copy(scr_bf[0:1, 1:2], diag_sb[0:1, 0:1]))
            rebase(diag_sb, td)

            tables = [nodec_d, pathc_d, nodec_d]
            # one shared Pool register per distinct gather size
            size_regs = {
                gch: nc.gpsimd.to_reg(gch * P) for gch in sorted(set(GROUPS))
            }

            # static gather destination tiles, one per (table, group):
            # [128 features, group columns] bf16, written once each
            gtiles = [
                [
                    gpool.tile(
                        [P, GROUPS[g] * P], bf16, name=f"g{t}_{g}", bufs=1
                    )
                    for g in range(NGROUP)
                ]
                for t in range(3)
            ]

            pending = []  # (k, da_tile, ht_tile) awaiting their wsum matmuls

            def emit_wsum(k, da_t, ht_t):
                chain(nc.tensor.matmul(
                    out=o_ps[:],
                    lhsT=da_t[:],
                    rhs=ht_t[:],
                    start=(k == 0),
                    stop=(k == NCHUNK - 1),
                ))
                chain(nc.tensor.matmul(
                    out=oz_ps[:],
                    lhsT=da_t[:],
                    rhs=ones_sb[:],
                    start=(k == 0),
                    stop=(k == NCHUNK - 1),
                    skip_group_check=True,
                ))

            o_ps = po_pool.tile([BPC, CV], f32, name="o_ps")
            oz_ps = po_pool.tile([BPC, 1], f32, name="oz_ps")

            call_no = 0
            for g in range(NGROUP):
                gch = GROUPS[g]
                # one dma_gather per table, transposed on the fly so SBUF
                # receives [feature, column] = ready-made matmul lhsT.
                # single_packet=False: transpose gathers with >=1024 indices
                # crash the exec unit in single-packet mode (probed on HW).
                # Queues rotate so desc-gen runs on different Q7 core pairs.
                for t in range(3):
                    gt = gtiles[t][g]
                    chain(nc.gpsimd.dma_gather(
                        gt[:].rearrange("p (a n) -> p a n", a=1),
                        tables[t][:],
                        idx_sb[:, t, g, 0:IDXW_G[g]],
                        gch * P,
                        size_regs[gch],
                        E,
                        transpose=True,
                        single_packet=False,
                        queue_num=call_no % NQ,
                    ))
                    call_no += 1
                # absorb the three gather-completion sems on PE, then
                # rebase so the matmuls below carry no gather waits
                for t in range(3):
                    ti = pe_touch(gtiles[t][g][0:1, 0:1])
                    rebase(gtiles[t][g], ti)

                # process the group in exp-batches of <= EB chunks
                for b0 in range(0, gch, EB):
                    bch = min(EB, gch - b0)
                    batch = []
                    for j in range(b0, b0 + bch):
                        k = GOFF[g] + j
                        h_ps = ph_pool.tile(
                            [P, CV], f32, name="h_ps", tag="h_ps", bufs=3
                        )
                        for t in range(3):
                            chain(nc.tensor.matmul(
                                out=h_ps[:],
                                lhsT=gtiles[t][g][:, j * P:(j + 1) * P],
                                rhs=wt_sb[:, t, :],
                                start=(t == 0),
                                stop=(t == 2),
                            ))
                        ht_t = wpool.tile(
                            [P, CV], bf16, name="ht", tag="ht", bufs=2 * EB + 4
                        )
                        chain(nc.scalar.activation(
                            out=ht_t[:], in_=h_ps[:],
                            func=mybir.ActivationFunctionType.Tanh,
                        ))
                        ttr_scratch = wpool.tile(
                            [P, CV], bf16, name="ttr_s", tag="ttr_s", bufs=3
                        )
                        # custom-DVE fused multiply+reduce:
                        #   out = ht*a, scores[:, k] = 0 + sum(out)
                        chain(nc.vector._custom_dve(
                            TENSOR_TENSOR_REDUCE,
                            out=ttr_scratch[:],
                            in0=ht_t[:],
                            in1=abc_sb[:],
                            s0=0.0,
                            s1=1.0,
                            accum_out=scores[:, k:k + 1],
                        ))
                        batch.append((k, ht_t))
                    # previous batch's weighted sums go to PE while this
                    # batch's scores chain through ACT/DVE
                    for item in pending:
                        emit_wsum(*item)
                    pending = []
                    # one exp per batch; scores are O(1) so no max pass
                    k0, k1 = batch[0][0], batch[-1][0] + 1
                    chain(nc.scalar.activation(
                        out=escore[:, k0:k1], in_=scores[:, k0:k1],
                        func=mybir.ActivationFunctionType.Exp,
                    ))


                    # da[p, b] = diag[p, b] * escore[p, k] on ACT (walrus
                    # allows multi-wait on InstActivation, unlike the DVE
                    # tensor-scalar struct, and the wsum matmul's single ACT
                    # wait then transitively covers tanh(k) as well)
                    for k, ht_t in batch:
                        da_t = wpool.tile(
                            [P, BPC], bf16, name="da", tag="da",
                            bufs=2 * EB + 4,
                        )
                        chain(nc.scalar.mul(
                            da_t[:], diag_sb[:], escore[:, k:k + 1]
                        ))
                        pending.append((k, da_t, ht_t))

            for item in pending:
                emit_wsum(*item)

            # ---- epilogue: out = o_ps / Z ----
            zsb = cpool.tile([BPC, 1], f32, name="zsb")
            chain(nc.vector.tensor_copy(zsb[:], oz_ps[:]))
            zinv = cpool.tile([BPC, 1], f32, name="zinv")
            chain(nc.vector.reciprocal(zinv[:], zsb[:]))
            out_sb = cpool.tile([BPC, CV], f32, name="out_sb")
            chain(nc.vector.tensor_scalar_mul(out_sb[:], o_ps[:], zinv[:]))

            pre_out_nops = [
                chain(nc.sync.nop(nofuse=True)).ins.name for _ in range(3)
            ]
            out_dma_name = chain(
                nc.sync.dma_start(out=out_d[:], in_=out_sb[:])
            ).ins.name

            # markers for the kernel-tail drain's wait redistribution
            tail_nops = [
                chain(nc.sync.nop(nofuse=True)).ins.name for _ in range(24)
            ]

    # Raw Bass (no Bacc.compile) never renders .instr bytes for InstISA
    # subclasses (the custom-DVE score op); render them here.
    mybir.codegen_inst_isa_subclasses(nc)

    # Tile sometimes emits *self-engine* sem waits (an instruction waiting
    # on its own engine's completion counter, e.g. the SWDGE ring gate on
    # PE, or dep-rebased const reads on DVE).  Engines execute in order, so
    # these waits are implied by program order; strip them when they would
    # push an instruction over walrus's sync-wait budget (1 for
    # Matmult/Ldweights/custom-DVE structs).
    fn = nc.m.functions[0]
    for blk in fn.blocks:
        for inst in blk.instructions:
            si = inst.sync_info
            if si is None or inst.engine is None:
                continue
            waits = list(si.on_wait)
            if len(waits) >= 2:
                own = inst.engine.name + "_"
                kept = [w for w in waits if not w.ant_name.startswith(own)]
                if len(kept) < len(waits):
                    si.on_wait = kept

    # The kernel-tail drain waits on every engine/DMA sem at once; walrus
    # allows one sync wait per instruction.  Redistribute its waits onto
    # the chained SP marker nops emitted just before it (SP executes them
    # in order, so waiting one sem per nop is equivalent).
    marker_set = set(tail_nops)
    pre_out_set = set(pre_out_nops)
    markers = []
    pre_out_markers = []
    drains = []
    out_dma = None
    for blk in fn.blocks:
        for inst in blk.instructions:
            if inst.name in marker_set:
                markers.append(inst)
            if inst.name in pre_out_set:
                pre_out_markers.append(inst)
            if inst.name == out_dma_name:
                out_dma = inst
            si = inst.sync_info
            if (
                type(inst).__name__ == "InstDrain"
                and si is not None
                and len(si.on_wait) > 1
            ):
                drains.append(inst)
    if out_dma is not None and len(out_dma.sync_info.on_wait) > 1:
        waits = list(out_dma.sync_info.on_wait)
        extra, last = waits[:-1], waits[-1:]
        assert len(extra) <= len(pre_out_markers)
        for w, m in zip(extra, pre_out_markers):
            msi = m.sync_info
            if msi is None:
                m.sync_info = mybir.SyncInfo(on_wait=[], on_update=[])
                msi = m.sync_info
            msi.on_wait = list(msi.on_wait) + [w]
        out_dma.sync_info.on_wait = last
    for drain in drains:
        si = drain.sync_info
        waits = list(si.on_wait)
        extra, last = waits[:-1], waits[-1:]
        assert len(extra) <= len(markers), (len(extra), len(markers))
        for w, m in zip(extra, markers):
            msi = m.sync_info
            if msi is None:
                m.sync_info = mybir.SyncInfo(on_wait=[], on_update=[])
                msi = m.sync_info
            msi.on_wait = list(msi.on_wait) + [w]
        markers = markers[len(extra):]
        si.on_wait = last

    return nc


def _host_prep(inputs):
    """Build per-core in_maps: usage-sharded bf16 tables + int16 indices."""
    starts = np.asarray(inputs["starts"])
    paths = np.asarray(inputs["paths"])
    ends = np.asarray(inputs["ends"])
    node_emb = np.asarray(inputs["node_emb"], dtype=np.float32)
    path_emb = np.asarray(inputs["path_emb"], dtype=np.float32)
    W = np.asarray(inputs["W"], dtype=np.float32)
    a = np.asarray(inputs["a"], dtype=np.float32)

    bf16 = ml_dtypes.bfloat16
    # wt[f, v] = W[v, f]  (h = ctx @ W.T)
    wt = np.ascontiguousarray(W.T).astype(bf16)
    a_bc = np.ascontiguousarray(
        np.broadcast_to(a[:, 0], (P, CV))
    ).astype(bf16)
    diag = np.zeros((P, BPC), dtype=bf16)
    for p in range(P):
        diag[p, p % BPC] = 1.0
    ones = np.ones((P, 1), dtype=bf16)

    in_maps = []
    for core in range(NCORES):
        b0 = core * BPC
        # column j = c*BPC + b  (c-major)
        flat = [
            src[b0:b0 + BPC].astype(np.int64).T.reshape(-1)
            for src in (starts, paths, ends)
        ]
        un = np.unique(np.concatenate((flat[0], flat[2])))
        upth = np.unique(flat[1])
        nodec = np.zeros((UN, E), dtype=bf16)
        nodec[:len(un)] = node_emb[un].astype(bf16)
        pathc = np.zeros((UP, E), dtype=bf16)
        pathc[:len(upth)] = path_emb[upth].astype(bf16)

        # idx[p, t, g, w]: index j (column w*16 + p%16 of group g) of table
        # t, replicated across the 8 GPSIMD cores' 16-partition stripes
        # (queue q's desc-gen cores read the replica in their own stripe)
        idx = np.zeros((P, 3, NGROUP, IDXW), dtype=np.int16)
        for t, (f, u) in enumerate(
            ((flat[0], un), (flat[1], upth), (flat[2], un))
        ):
            loc = np.searchsorted(u, f).astype(np.int16)  # [R]
            for g in range(NGROUP):
                seg = loc[GOFF[g] * P:(GOFF[g] + GROUPS[g]) * P]
                blk = seg.reshape(IDXW_G[g], 16).T  # [16, w]
                idx[:, t, g, 0:IDXW_G[g]] = np.tile(blk, (8, 1))
        in_maps.append({
            "nodec": nodec,
            "pathc": pathc,
            "wt": wt,
            "a_bc": a_bc,
            "diag": diag,
            "ones": ones,
            "idx": np.ascontiguousarray(idx),
        })
    return in_maps


def _get_program():
    if "nc" not in _compiled:
        _compiled["nc"] = _build_program()
    return _compiled["nc"]


def run_on_device(inputs, trace=False, **kwargs):
    """Run the SPMD kernel; returns (full_output, BassKernelResults)."""
    from concourse import bass_utils

    nc = _get_program()
    in_maps = _host_prep(inputs)
    res = bass_utils.run_bass_kernel_spmd(
        nc, in_maps, core_ids=list(range(NCORES)), trace=trace, **kwargs
    )
    out = np.concatenate(
        [res.results[c]["out"] for c in range(NCORES)], axis=0
    ).astype(np.float32)
    return out, res


def kernel(**inputs) -> np.ndarray:
    out, _ = run_on_device(inputs, trace=False)
    return out


# revision 25
# speedup vs baseline: 1.0183x; 1.0183x over previous
"""Trainium2 Bass kernel for a Code2Vec encoder.

Reference computation (per batch b of 512, contexts c of 200):
    s = node_emb[starts]; p = path_emb[paths]; e = node_emb[ends]
    ctx = concat(s, p, e)                      # [B, C, 384]
    h = tanh(ctx @ W.T)                        # [B, C, 384]
    scores = h @ a                             # [B, C, 1]
    attn = softmax(scores, axis=1)
    out = sum(h * attn, axis=1)                # [B, 384]

Sharding: data-parallel over batch across 8 NeuronCores (64 batches each).
Embedding tables are row-sharded per core by usage: each core's in_map holds
only the table rows that core actually references (<=25600 node rows,
<=12800 path rows per core), cast to bf16, with indices remapped to int16
local ids.  That bounds indices to the int16 range dma_gather requires.

Per-core layout: columns j = c*64 + b (c-major), 12800 columns in 100
chunks of 128.  Chunk k, partition-within-chunk p <-> (b = p % 64,
c = 2k + p // 64).
 - gather s/p/e via SWDGE dma_gather(transpose=True): one call per
   (table, group of 2560 columns) batches 2560 descriptors (~1.9us Pool
   prep each vs ~1us per 128 rows for indirect_dma_start) AND lands the
   data transposed in SBUF as [128 features, columns] = ready-made lhsT.
 - matmul per 128-column chunk (lhsT = gathered chunk per table, rhs =
   W.T rows, all bf16) accumulating the three 128-feature groups ->
   psum h [128 cols, 384v]
 - ACT tanh -> bf16 ht chunk; DVE tensor_tensor_reduce ht*a -> scores col
 - attention without the max-subtraction pass (scores are O(1), exp is
   safe): escore = exp(scores) on ACT, da[p, b] = diag[p, b]*escore[p]
   on ACT, then two PE matmuls accumulate o_ps[b, v] += da^T @ ht and
   oz_ps[b] += da^T @ 1 across all 100 chunks.  wsum matmuls trail the
   main loop by LAG chunks so PE never stalls on the ACT/DVE score chain.
 - epilogue: out = o_ps * reciprocal(oz_ps)  (the softmax normalization)
"""

import os
import numpy as np
import ml_dtypes

B, C, E = 512, 200, 128
NODES, PATHS, CV = 100000, 1000000, 384
NCORES = 8
BPC = B // NCORES          # 64 batches per core
R = C * BPC                # 12800 columns per core
P = 128
NCHUNK = R // P            # 100 chunks of 128 columns
# Gather group sizes in chunks: small first (fast pipeline fill), big in
# the middle (amortize the ~1us SWDGE launch per call), small last (short
# PE tail after the final gather).  All gathers stay on ONE SWDGE queue:
# transpose-mode gathers go through the shared XBAR, and concurrent
# transpose gathers on different queues corrupt each other (probed on HW).
# (28-chunk groups were tried and regress ~7%: 3584-descriptor calls stall
# the Q7 desc-gen against ring drainage; <=24 chunks overlaps cleanly.)
GROUPS = (2, 24, 24, 24, 24, 2)
assert sum(GROUPS) == NCHUNK
NGROUP = len(GROUPS)
GOFF = [sum(GROUPS[:i]) for i in range(NGROUP)]  # first chunk of each group
GMAX = max(GROUPS)
IDXW_G = [g * P // 16 for g in GROUPS]           # idx words per group
IDXW = GMAX * P // 16                            # idx tile pitch (padded)
NQ = 1                     # SWDGE queues; queue q runs on Q7 cores (2q,2q+1)
UN = 2 * R                 # compact node table rows (worst case 25600)
UP = R                     # compact path table rows (worst case 12800)
EB = 10                    # exp batch: one ACT exp per EB chunks; wsum
                           # matmuls trail the score chain by one batch

_compiled = {}


def _build_program():
    import concourse.bass as bass
    import concourse.mybir as mybir
    import concourse.tile as tile
    from concourse.tile_rust import add_dep_helper
    from concourse.dve_ops import TENSOR_TENSOR_REDUCE
    from concourse.library_config import mlp

    f32 = mybir.dt.float32
    bf16 = mybir.dt.bfloat16
    i16 = mybir.dt.int16

    # 64 KiB SWDGE descriptor-ring carveout (default 16 KiB = 1024 descs
    # per queue; each 2560-column dma_gather pushes 2560 descriptors, and an
    # entry that can never fit wedges the Q7 desc-gen kernel forever).
    nc = bass.Bass(
        "TRN2", target_bir_lowering=False, debug=False,
        dynamic_dma_scratch_size=65536,
        num_swdge_queues=NQ,
    )

    # Suppress Tile's tail DMA-reset drain (19 sem waits — walrus allows 1
    # per instruction) and sem range-clear.  They only matter for relaunch
    # of the same loaded NEFF; every kernel() call loads a fresh NEFF.
    type(nc.gpsimd).dma_reset = lambda self, *a, **k: None
    type(nc.gpsimd).sem_clear = lambda self, *a, **k: None

    # Per-engine emission-order chaining (free same-engine ordering edges).
    # This pins the per-engine schedule to emission order so the
    # single-sync-wait budget of Matmult/Ldweights is respected by
    # construction: each instruction's older cross-engine deps are already
    # covered by the engine clock via earlier chained instructions.
    _prev = {}

    def chain(bi):
        eng = bi.ins.engine
        p = _prev.get(eng)
        if p is not None:
            add_dep_helper(bi.ins, p.ins, sync=False, reason="engine chain")
        _prev[eng] = bi
        return bi

    # DRAM I/O.  Compact per-core tables (bf16), padded to fixed shapes.
    nodec_d = nc.dram_tensor("nodec", [UN, E], bf16, kind="ExternalInput").ap()
    pathc_d = nc.dram_tensor("pathc", [UP, E], bf16, kind="ExternalInput").ap()
    wt_d = nc.dram_tensor("wt", [3 * E, CV], bf16, kind="ExternalInput").ap()
    abc_d = nc.dram_tensor("a_bc", [P, CV], bf16, kind="ExternalInput").ap()
    diag_d = nc.dram_tensor("diag", [P, BPC], bf16, kind="ExternalInput").ap()
    ones_d = nc.dram_tensor("ones", [P, 1], bf16, kind="ExternalInput").ap()
    idx_d = nc.dram_tensor(
        "idx", [P, 3, NGROUP, IDXW], i16, kind="ExternalInput"
    ).ap()
    out_d = nc.dram_tensor("out", [BPC, CV], f32, kind="ExternalOutput").ap()

    with tile.TileContext(nc) as tc:
        with (
            tc.tile_pool(name="const", bufs=1) as cpool,
            tc.tile_pool(name="gather", bufs=1) as gpool,
            tc.tile_pool(name="work", bufs=4) as wpool,
            tc.tile_pool(name="psum_h", bufs=3, space="PSUM") as ph_pool,
            tc.tile_pool(name="psum_o", bufs=1, space="PSUM") as po_pool,
        ):
            # dma_gather runs as a GPSIMD Q7 ucode kernel from the `mlp`
            # library; load it before the first gather.  (No other Pool
            # instruction in this program needs a different library.)
            chain(nc.gpsimd.load_library(mlp))

            # ---- constants ----
            wt_sb = cpool.tile([P, 3, CV], bf16, name="wt_sb")
            chain(nc.sync.dma_start(
                out=wt_sb[:], in_=wt_d.rearrange("(t p) v -> p t v", p=P)
            ))
            abc_sb = cpool.tile([P, CV], bf16, name="abc_sb")
            chain(nc.sync.dma_start(out=abc_sb[:], in_=abc_d[:]))
            diag_sb = cpool.tile([P, BPC], bf16, name="diag_sb")
            chain(nc.sync.dma_start(out=diag_sb[:], in_=diag_d[:]))
            ones_sb = cpool.tile([P, 1], bf16, name="ones_sb")
            chain(nc.sync.dma_start(out=ones_sb[:], in_=ones_d[:]))
            idx_sb = cpool.tile([P, 3, NGROUP, IDXW], i16, name="idx_sb")
            chain(nc.sync.dma_start(out=idx_sb[:], in_=idx_d[:]))

            # score columns (tanh outputs live in a short ring of chunk
            # tiles: the wsum matmul consumes chunk k only LAG chunks later,
            # and the 64 KiB DMA-scratch carveout wants the SBUF back)
            scores = cpool.tile([P, NCHUNK], f32, name="scores")
            escore = cpool.tile([P, NCHUNK], f32, name="escore")

            # One-time "touch" reads absorb the const-DMA waits on each
            # consuming engine (walrus allows only ONE sync wait on a
            # Matmult/Ldweights, so no PE instruction may need two).
            scr_ps = po_pool.tile([1, 8], f32, name="scr_ps")
            scr_sb = cpool.tile([1, 8], f32, name="scr_sb")
            scr_bf = cpool.tile([1, 8], bf16, name="scr_bf")

            def pe_touch(ap):
                return chain(nc.tensor.matmul(
                    out=scr_ps[0:1, 0:1], lhsT=ap, rhs=ap,
                    start=True, stop=True, skip_group_check=True,
                ))

            def rebase(tile_h, inst):
                # Rebase a tile's dep state onto `inst` so every later
                # reader deps on a same-engine instruction (no extra sem
                # wait) instead of the original DMA.
                tc.dep_state.clear_tensor_accesses(tile_h.tensor.name)
                tc.dep_state.set_after_insts(tile_h.tensor.name, inst.ins)

            tw = pe_touch(wt_sb[0:1, 0, 0:1])
            rebase(wt_sb, tw)
            to = pe_touch(ones_sb[0:1, 0:1])
            rebase(ones_sb, to)
            ta = chain(nc.vector.tensor_copy(scr_bf[0:1, 0:1], abc_sb[0:1, 0:1]))
            rebase(abc_sb, ta)
            td = chain(nc.scalar.copy(scr_bf[0:1, 1:2], diag_sb[0:1, 0:1]))
            rebase(diag_sb, td)

            tables = [nodec_d, pathc_d, nodec_d]
            # one shared Pool register per distinct gather size
            size_regs = {
                gch: nc.gpsimd.to_reg(gch * P) for gch in sorted(set(GROUPS))
            }

            # static gather destination tiles, one per (table, group):
            # [128 features, group columns] bf16, written once each
            gtiles = [
                [
                    gpool.tile(
                        [P, GROUPS[g] * P], bf16, name=f"g{t}_{g}", bufs=1
                    )
                    for g in range(NGROUP)
                ]
                for t in range(3)
            ]

            pending = []  # (k, da_tile, ht_tile) awaiting their wsum matmuls

            def emit_wsum(k, da_t, ht_t):
                chain(nc.tensor.matmul(
                    out=o_ps[:],
                    lhsT=da_t[:],
                    rhs=ht_t[:],
                    start=(k == 0),
                    stop=(k == NCHUNK - 1),
                ))
                chain(nc.tensor.matmul(
                    out=oz_ps[:],
                    lhsT=da_t[:],
                    rhs=ones_sb[:],
                    start=(k == 0),
                    stop=(k == NCHUNK - 1),
                    skip_group_check=True,
                ))

            o_ps = po_pool.tile([BPC, CV], f32, name="o_ps")
            oz_ps = po_pool.tile([BPC, 1], f32, name="oz_ps")

            call_no = 0
            for g in range(NGROUP):
                gch = GROUPS[g]
                # one dma_gather per table, transposed on the fly so SBUF
                # receives [feature, column] = ready-made matmul lhsT.
                # single_packet=False: transpose gathers with >=1024 indices
                # crash the exec unit in single-packet mode (probed on HW).
                # Queues rotate so desc-gen runs on different Q7 core pairs.
                for t in range(3):
                    gt = gtiles[t][g]
                    chain(nc.gpsimd.dma_gather(
                        gt[:].rearrange("p (a n) -> p a n", a=1),
                        tables[t][:],
                        idx_sb[:, t, g, 0:IDXW_G[g]],
                        gch * P,
                        size_regs[gch],
                        E,
                        transpose=True,
                        single_packet=False,
                        queue_num=call_no % NQ,
                    ))
                    call_no += 1
                # absorb the three gather-completion sems on PE, then
                # rebase so the matmuls below carry no gather waits
                for t in range(3):
                    ti = pe_touch(gtiles[t][g][0:1, 0:1])
                    rebase(gtiles[t][g], ti)

                # process the group in exp-batches of <= EB chunks
                for b0 in range(0, gch, EB):
                    bch = min(EB, gch - b0)
                    batch = []
                    for j in range(b0, b0 + bch):
                        k = GOFF[g] + j
                        h_ps = ph_pool.tile(
                            [P, CV], f32, name="h_ps", tag="h_ps", bufs=3
                        )
                        for t in range(3):
                            chain(nc.tensor.matmul(
                                out=h_ps[:],
                                lhsT=gtiles[t][g][:, j * P:(j + 1) * P],
                                rhs=wt_sb[:, t, :],
                                start=(t == 0),
                                stop=(t == 2),
                            ))
                        ht_t = wpool.tile(
                            [P, CV], bf16, name="ht", tag="ht", bufs=2 * EB + 4
                        )
                        chain(nc.scalar.activation(
                            out=ht_t[:], in_=h_ps[:],
                            func=mybir.ActivationFunctionType.Tanh,
                        ))
                        ttr_scratch = wpool.tile(
                            [P, CV], bf16, name="ttr_s", tag="ttr_s", bufs=3
                        )
                        # custom-DVE fused multiply+reduce:
                        #   out = ht*a, scores[:, k] = 0 + sum(out)
                        chain(nc.vector._custom_dve(
                            TENSOR_TENSOR_REDUCE,
                            out=ttr_scratch[:],
                            in0=ht_t[:],
                            in1=abc_sb[:],
                            s0=0.0,
                            s1=1.0,
                            accum_out=scores[:, k:k + 1],
                        ))
                        batch.append((k, ht_t))
                    # previous batch's weighted sums go to PE while this
                    # batch's scores chain through ACT/DVE
                    for item in pending:
                        emit_wsum(*item)
                    pending = []
                    # one exp per batch; scores are O(1) so no max pass
                    k0, k1 = batch[0][0], batch[-1][0] + 1
                    chain(nc.scalar.activation(
                        out=escore[:, k0:k1], in_=scores[:, k0:k1],
                        func=mybir.ActivationFunctionType.Exp,
                    ))


                    # da[p, b] = diag[p, b] * escore[p, k] on ACT (walrus
                    # allows multi-wait on InstActivation, unlike the DVE
                    # tensor-scalar struct, and the wsum matmul's single ACT
                    # wait then transitively covers tanh(k) as well)
                    for k, ht_t in batch:
                        da_t = wpool.tile(
                            [P, BPC], bf16, name="da", tag="da",
                            bufs=2 * EB + 4,
                        )
                        chain(nc.scalar.mul(
                            da_t[:], diag_sb[:], escore[:, k:k + 1]
                        ))
                        pending.append((k, da_t, ht_t))

            for item in pending:
                emit_wsum(*item)

            # ---- epilogue: out = o_ps / Z ----
            zsb = cpool.tile([BPC, 1], f32, name="zsb")
            chain(nc.vector.tensor_copy(zsb[:], oz_ps[:]))
            zinv = cpool.tile([BPC, 1], f32, name="zinv")
            chain(nc.vector.reciprocal(zinv[:], zsb[:]))
            out_sb = cpool.tile([BPC, CV], f32, name="out_sb")
            chain(nc.vector.tensor_scalar_mul(out_sb[:], o_ps[:], zinv[:]))

            pre_out_nops = [
                chain(nc.sync.nop(nofuse=True)).ins.name for _ in range(3)
            ]
            out_dma_name = chain(
                nc.sync.dma_start(out=out_d[:], in_=out_sb[:])
            ).ins.name

            # markers for the kernel-tail drain's wait redistribution
            tail_nops = [
                chain(nc.sync.nop(nofuse=True)).ins.name for _ in range(24)
            ]

    # Raw Bass (no Bacc.compile) never renders .instr bytes for InstISA
    # subclasses (the custom-DVE score op); render them here.
    mybir.codegen_inst_isa_subclasses(nc)

    # Tile sometimes emits *self-engine* sem waits (an instruction waiting
    # on its own engine's completion counter, e.g. the SWDGE ring gate on
    # PE, or dep-rebased const reads on DVE).  Engines execute in order, so
    # these waits are implied by program order; strip them when they would
    # push an instruction over walrus's sync-wait budget (1 for
    # Matmult/Ldweights/custom-DVE structs).
    fn = nc.m.functions[0]
    for blk in fn.blocks:
        for inst in blk.instructions:
            si = inst.sync_info
            if si is None or inst.engine is None:
                continue
            waits = list(si.on_wait)
            if len(waits) >= 2:
                own = inst.engine.name + "_"
                kept = [w for w in waits if not w.ant_name.startswith(own)]
                if len(kept) < len(waits):
                    si.on_wait = kept

    # The kernel-tail drain waits on every engine/DMA sem at once; walrus
    # allows one sync wait per instruction.  Redistribute its waits onto
    # the chained SP marker nops emitted just before it (SP executes them
    # in order, so waiting one sem per nop is equivalent).
    marker_set = set(tail_nops)
    pre_out_set = set(pre_out_nops)
    markers = []
    pre_out_markers = []
    drains = []
    out_dma = None
    for blk in fn.blocks:
        for inst in blk.instructions:
            if inst.name in marker_set:
                markers.append(inst)
            if inst.name in pre_out_set:
                pre_out_markers.append(inst)
            if inst.name == out_dma_name:
                out_dma = inst
            si = inst.sync_info
            if (
                type(inst).__name__ == "InstDrain"
                and si is not None
                and len(si.on_wait) > 1
            ):
                drains.append(inst)
    if out_dma is not None and len(out_dma.sync_info.on_wait) > 1:
        waits = list(out_dma.sync_info.on_wait)
        extra, last = waits[:-1], waits[-1:]
        assert len(extra) <= len(pre_out_markers)
        for w, m in zip(extra, pre_out_markers):
            msi = m.sync_info
            if msi is None:
                m.sync_info = mybir.SyncInfo(on_wait=[], on_update=[])
                msi = m.sync_info
            msi.on_wait = list(msi.on_wait) + [w]
        out_dma.sync_info.on_wait = last
    for drain in drains:
        si = drain.sync_info
        waits = list(si.on_wait)
        extra, last = waits[:-1], waits[-1:]
        assert len(extra) <= len(markers), (len(extra), len(markers))
        for w, m in zip(extra, markers):
            msi = m.sync_info
            if msi is None:
                m.sync_info = mybir.SyncInfo(on_wait=[], on_update=[])
                msi = m.sync_info
            msi.on_wait = list(msi.on_wait) + [w]
        markers = markers[len(extra):]
        si.on_wait = last

    return nc


def _host_prep(inputs):
    """Build per-core in_maps: usage-sharded bf16 tables + int16 indices."""
    starts = np.asarray(inputs["starts"])
    paths = np.asarray(inputs["paths"])
    ends = np.asarray(inputs["ends"])
    node_emb = np.asarray(inputs["node_emb"], dtype=np.float32)
    path_emb = np.asarray(inputs["path_emb"], dtype=np.float32)
    W = np.asarray(inputs["W"], dtype=np.float32)
    a = np.asarray(inputs["a"], dtype=np.float32)

    bf16 = ml_dtypes.bfloat16
    # wt[f, v] = W[v, f]  (h = ctx @ W.T)
    wt = np.ascontiguousarray(W.T).astype(bf16)
    a_bc = np.ascontiguousarray(
        np.broadcast_to(a[:, 0], (P, CV))
    ).astype(bf16)
    diag = np.zeros((P, BPC), dtype=bf16)
    for p in range(P):
        diag[p, p % BPC] = 1.0
    ones = np.ones((P, 1), dtype=bf16)

    in_maps = []
    for core in range(NCORES):
        b0 = core * BPC
        # column j = c*BPC + b  (c-major)
        flat = [
            src[b0:b0 + BPC].astype(np.int64).T.reshape(-1)
            for src in (starts, paths, ends)
        ]
        un = np.unique(np.concatenate((flat[0], flat[2])))
        upth = np.unique(flat[1])
        nodec = np.zeros((UN, E), dtype=bf16)
        nodec[:len(un)] = node_emb[un].astype(bf16)
        pathc = np.zeros((UP, E), dtype=bf16)
        pathc[:len(upth)] = path_emb[upth].astype(bf16)

        # idx[p, t, g, w]: index j (column w*16 + p%16 of group g) of table
        # t, replicated across the 8 GPSIMD cores' 16-partition stripes
        # (queue q's desc-gen cores read the replica in their own stripe)
        idx = np.zeros((P, 3, NGROUP, IDXW), dtype=np.int16)
        for t, (f, u) in enumerate(
            ((flat[0], un), (flat[1], upth), (flat[2], un))
        ):
            loc = np.searchsorted(u, f).astype(np.int16)  # [R]
            for g in range(NGROUP):
                seg = loc[GOFF[g] * P:(GOFF[g] + GROUPS[g]) * P]
                blk = seg.reshape(IDXW_G[g], 16).T  # [16, w]
                idx[:, t, g, 0:IDXW_G[g]] = np.tile(blk, (8, 1))
        in_maps.append({
            "nodec": nodec,
            "pathc": pathc,
            "wt": wt,
            "a_bc": a_bc,
            "diag": diag,
            "ones": ones,
            "idx": np.ascontiguousarray(idx),
        })
    return in_maps


def _get_program():
    if "nc" not in _compiled:
        _compiled["nc"] = _build_program()
    return _compiled["nc"]


def run_on_device(inputs, trace=False, **kwargs):
    """Run the SPMD kernel; returns (full_output, BassKernelResults)."""
    from concourse import bass_utils

    nc = _get_program()
    in_maps = _host_prep(inputs)
    res = bass_utils.run_bass_kernel_spmd(
        nc, in_maps, core_ids=list(range(NCORES)), trace=trace, **kwargs
    )
    out = np.concatenate(
        [res.results[c]["out"] for c in range(NCORES)], axis=0
    ).astype(np.float32)
    return out, res


def kernel(**inputs) -> np.ndarray:
    out, _ = run_on_device(inputs, trace=False)
    return out
